# revision 24
# baseline (speedup 1.0000x reference)
"""Trainium2 Bass kernel for nn_ADS_30313879175331.

Pipeline (reference):
  attn-softmax pooling over T -> x *= (1+aw) -> shuffle tokens by perm
  -> Linear(D,D)+GELU -> rearrange (B,T/4,4,D)->(B,T/4,D*4)
  -> gather keep_idx columns -> Linear(D,D) -> (B, T/4, D)

Device strategy (8 cores, PAIR-sharded):
  * Core c handles batch b=c//2, permuted-token half h=c%2 (8192 tokens),
    i.e. output rows [h*2048, (h+1)*2048) of batch b.  The softmax
    denominator for batch b then only needs a 2-core AllReduce within the
    pair (groups [[0,1],[2,3],[4,5],[6,7]]) instead of 4 serialized 8-core
    AllReduces -- inter-core arrival skew no longer serializes the kernel.
  * Host folds perm + the (rearrange+keep_idx gather) into pure data layout:
    tokens grouped per (core, u-block k, class r = shuffled_pos % 4); embed
    weight columns {d : 4d+r in keep_idx} and matching w_down rows are
    pre-selected per class, so the device kernel is fully dense.
  * Per (k,r) tile of 512 tokens (x stored transposed, d on partitions):
      attn1 matmul -> tanh -> logit matmul with w2 replicated over 128 cols
      -> Exp activation with fused row-sum -> embed matmul h = x @ We_r.
    The attention path reads a separate fp8 copy of x (half the DMA) and
    runs its matmuls in fp8 DoubleRow mode (2x PE rate; w1 pre-scaled by
    64 into fp8 range, compensated in the tanh's scale).  Attention is
    ordered FIRST within each iteration at 2 tiles/iter, so the single
    pair AllReduce (16 partial sums) triggers by ~45us and lands long
    before the embed matmuls finish.  Errors here only perturb the softmax
    logits (|logit| ~ 0.1) at ~5e-3 absolute, and aw itself is O(1e-4).
  * Stage 2: s = 1 + e/den ; g = gelu(h*s) ; out = sum_r g_r @ Wd_r + b_down,
    software-pipelined (gelu of k+1 overlaps down of k).
  All matmuls bf16 with f32 PSUM accumulation.
"""

import numpy as np
import ml_dtypes

B, T, D, ATTN, R = 4, 16384, 1024, 128, 4
N_CORES = 8
K = 4                       # u-blocks per core = 4 x 512 rows = 2048 rows
U = 512                     # tokens per (k,r) tile / output rows per block
DC = D // 128               # contraction chunks over D = 8
P = 128
NT = K * R                  # 16 tiles per core
XPOOL = 6                   # bf16 x tile buffers resident in SBUF (embed)
X8POOL = 6                  # fp8 x tile buffers (attention)
W1S = 64.0                  # fp8 range pre-scale for w_attn1

_BF16 = ml_dtypes.bfloat16
_FP8 = ml_dtypes.float8_e4m3fn


def _host_prep(x, w_attn1, b_attn1, w_attn2, b_attn2,
               w_embed, b_embed, w_down, b_down, perm, keep_idx):
    """Pure-layout host work: sharding, permutation gather, weight selection."""
    perm = np.asarray(perm).astype(np.int64)
    keep = np.asarray(keep_idx).astype(np.int64)
    x = np.asarray(x, dtype=np.float32)

    # class split of keep_idx (duplicates preserved, order by j)
    cols, rows = [], []
    for r in range(R):
        sel = np.nonzero((keep % R) == r)[0]
        rows.append(sel)                  # indices j into w_down rows
        cols.append(keep[sel] // R)       # embed output columns d
    Kr = [len(c) for c in cols]
    KC = [(k + P - 1) // P for k in Kr]   # 128-chunks per class (may be 0)
    Kp = [kc * P for kc in KC]
    SKC = sum(KC)
    SKP = sum(Kp)
    OFFC = np.concatenate([[0], np.cumsum(KC)]).astype(int)  # chunk offsets

    f32 = np.float32
    we = np.zeros((D, SKP), dtype=f32)
    wd = np.zeros((SKP, D), dtype=f32)
    be = np.zeros((SKP,), dtype=f32)
    for r in range(R):
        o = OFFC[r] * P
        if Kr[r]:
            we[:, o:o + Kr[r]] = np.asarray(w_embed, f32)[:, cols[r]]
            wd[o:o + Kr[r], :] = np.asarray(w_down, f32)[rows[r], :]
            be[o:o + Kr[r]] = np.asarray(b_embed, f32)[cols[r]]
    # bias per (partition, chunk) layout for per-partition activation bias
    be_pc = be.reshape(SKC, P).T.copy()                       # (128, SKC)

    w1 = (np.asarray(w_attn1, f32) * W1S).astype(_FP8)        # (D, ATTN)
    w2r = np.tile(np.asarray(w_attn2, f32).reshape(ATTN, 1), (1, P)).astype(_BF16)
    b1 = np.asarray(b_attn1, f32).reshape(ATTN, 1)
    b2 = np.full((P, 1), float(np.asarray(b_attn2, f32).reshape(-1)[0]), f32)
    bd = np.broadcast_to(np.asarray(b_down, f32), (P, D)).astype(_BF16)

    # x gather per core: core c = (batch b=c//2, half h=c%2).
    # x_pre[c][k, r, d, u] = x[b, perm[h*8192 + k*2048 + 4u + r], d]
    pidx = perm.reshape(2, K, U, R)                           # [h, k, u, r]
    g = x[:, pidx, :]                                         # (B, 2, K, U, R, D)
    x_pre, x8_pre = [], []
    for c in range(N_CORES):
        arr = np.ascontiguousarray(
            g[c // 2, c % 2].transpose(0, 2, 3, 1))           # (K, R, D, U)
        x_pre.append(arr.astype(_BF16))
        x8_pre.append(arr.astype(_FP8))

    meta = dict(Kr=Kr, KC=KC, Kp=Kp, SKC=SKC, SKP=SKP, OFFC=OFFC,
                use_bd=bool(np.any(np.asarray(b_down))),
                use_be=bool(np.any(np.asarray(b_embed))))
    weights = dict(
        w1=w1, w2r=w2r, b1=b1, b2=b2, bd=bd,
        we=we.astype(_BF16), wd=wd.astype(_BF16), be=be_pc,
    )
    return x_pre, x8_pre, weights, meta


def _attn_schedule():
    """attn tiles handled at each embed iteration: 2/iter off the fp8
    stream (tiles 0..R-1 run in the prologue)."""
    sched = {t: [] for t in range(NT)}
    nxt = R
    for t in range(NT):
        cap = 2 if t < (NT - R) // 3 else 1
        while nxt < NT and len(sched[t]) < cap:
            sched[t].append(nxt)
            nxt += 1
    assert nxt == NT
    return sched


def _build(meta):
    import concourse.bacc as bacc
    import concourse.mybir as mybir
    import concourse.tile as tile

    dt = mybir.dt
    AF = mybir.ActivationFunctionType
    ALU = mybir.AluOpType
    KC, SKC, SKP, OFFC = meta["KC"], meta["SKC"], meta["SKP"], meta["OFFC"]
    USE_BD = meta["use_bd"]
    USE_BE = meta["use_be"]

    nc = bacc.Bacc(None, target_bir_lowering=False, debug=False,
                   num_devices=N_CORES)

    xp = nc.declare_dram_parameter("x", [K, R, D, U], dt.bfloat16, isOutput=False)
    x8p = nc.declare_dram_parameter("x8", [K, R, D, U], dt.float8e4, isOutput=False)
    w1p = nc.declare_dram_parameter("w1", [D, ATTN], dt.float8e4, isOutput=False)
    w2p = nc.declare_dram_parameter("w2r", [ATTN, P], dt.bfloat16, isOutput=False)
    wep = nc.declare_dram_parameter("we", [D, SKP], dt.bfloat16, isOutput=False)
    wdp = nc.declare_dram_parameter("wd", [SKP, D], dt.bfloat16, isOutput=False)
    bep = nc.declare_dram_parameter("be", [P, SKC], dt.float32, isOutput=False)
    b1p = nc.declare_dram_parameter("b1", [ATTN, 1], dt.float32, isOutput=False)
    b2p = nc.declare_dram_parameter("b2", [P, 1], dt.float32, isOutput=False)
    bdp = nc.declare_dram_parameter("bd", [P, D], dt.bfloat16, isOutput=False)
    outp = nc.declare_dram_parameter("out", [K, U, D], dt.float32, isOutput=True)

    sched = _attn_schedule()

    with tile.TileContext(nc) as tc:
        with (
            tc.tile_pool(name="const", bufs=1) as cpool,
            tc.tile_pool(name="xin", bufs=XPOOL) as xpool,
            tc.tile_pool(name="x8in", bufs=X8POOL) as x8pool,
            tc.tile_pool(name="gts", bufs=2) as gpool,
            tc.tile_pool(name="outs", bufs=2) as opool,
            tc.tile_pool(name="tmps", bufs=3) as tpool,
            tc.tile_pool(name="psA", bufs=2, space="PSUM") as psA,
            tc.tile_pool(name="psO", bufs=2, space="PSUM") as psO,  # 2-bank tiles
            tc.tile_pool(name="dram", bufs=1, space="DRAM") as dram,
        ):
            # ---- resident constants (attention weights + first x tile first
            # so attn starts ASAP; the rest behind) ----
            w1_sb = cpool.tile([P, DC, ATTN], dt.float8e4)
            nc.scalar.dma_start(w1_sb[:], w1p.ap().rearrange("(c p) a -> p c a", p=P))
            w2r_sb = cpool.tile([P, P], dt.bfloat16)
            nc.scalar.dma_start(w2r_sb[:], w2p[:, :])
            b1_sb = cpool.tile([ATTN, 1], dt.float32)
            nc.scalar.dma_start(b1_sb[:], b1p[:, :])
            b2_sb = cpool.tile([P, 1], dt.float32)
            nc.scalar.dma_start(b2_sb[:], b2p[:, :])
            be_sb = cpool.tile([P, SKC], dt.float32)
            we_sb = cpool.tile([P, DC, SKP], dt.bfloat16)
            _we_ap = wep.ap().rearrange("(c p) k -> p c k", p=P)

            def load_we(_r):
                _a, _b = OFFC[_r] * P, OFFC[_r + 1] * P
                if _a < _b:
                    nc.scalar.dma_start(we_sb[:, :, _a:_b], _we_ap[:, :, _a:_b])

            bd_sb = cpool.tile([P, D], dt.bfloat16)
            wd_sb = cpool.tile([P, SKC, D], dt.bfloat16)

            e_sb = cpool.tile([P, K, R, U], dt.float8e4)    # exp(logits), bcast rows
            esum_sb = cpool.tile([P, NT], dt.float32)       # per-(k,r) local sums
            h_sb = cpool.tile([P, K, SKC, U], dt.bfloat16)  # x @ We (transposed)
            inv_bc = cpool.tile([P, 1], dt.float32)         # 1/denominator bcast
            den_sb = cpool.tile([1, NT + 8], dt.float32)    # raw sums + reduced

            bounce_in = dram.tile([1, NT], dt.float32, name="cc_in")
            # NOTE: Shared outputs need >4-core groups; pairs use Local.
            bounce_out = dram.tile([1, NT], dt.float32, name="cc_out")

            xt_tiles, x8_tiles, aTs_tiles = {}, {}, {}

            def heater(n):
                # dummy matmuls on resident weights: keeps the PE HAM window
                # busy across startup DMA gaps so the clock stays at 2.4GHz
                hps = psA.tile([P, 64], dt.float32, tag="psAt")
                for i in range(n):
                    nc.tensor.matmul(hps[:], w2r_sb[:], w2r_sb[:, :64],
                                     start=(i == 0), stop=(i == n - 1))
                nc.vector.tensor_copy(den_sb[0:1, NT + 6:NT + 7], hps[0:1, 0:1])

            def load_x(t):
                k, r = divmod(t, R)
                xt = xpool.tile([P, DC, U], dt.bfloat16, tag="xt", name=f"xt{t}")
                src = xp[k, r].rearrange("(c p) u -> p c u", p=P)
                if t == 0:
                    for c in range(DC):
                        nc.sync.dma_start(xt[:, c], src[:, c])
                else:
                    nc.sync.dma_start(xt[:], src)
                xt_tiles[t] = xt

            def load_x8(t):
                # tiles >= R go on the (otherwise idle) gpsimd queue:
                # decoupled from the bf16 x stream so attention is never
                # starved behind it; first tiles use sync (gpsimd preamble
                # delays its first DMA by ~6us)
                k, r = divmod(t, R)
                xt = x8pool.tile([P, DC, U], dt.float8e4, tag="x8", name=f"x8t{t}")
                src = x8p[k, r].rearrange("(c p) u -> p c u", p=P)
                eng = nc.sync if t < R else nc.gpsimd
                if t == 0:
                    for c in range(DC):
                        eng.dma_start(xt[:, c], src[:, c])
                else:
                    eng.dma_start(xt[:], src)
                x8_tiles[t] = xt

            def attn1_tile(t):
                # fp8 DoubleRow: contraction 256 per instruction, 2x PE rate
                xt = x8_tiles.pop(t)
                aT = psA.tile([P, U], dt.float32, tag="psAt")
                for c in range(DC // 2):
                    nc.tensor.matmul(aT[:], w1_sb[:, 2 * c:2 * c + 2, :],
                                     xt[:, 2 * c:2 * c + 2, :],
                                     start=(c == 0), stop=(c == DC // 2 - 1),
                                     perf_mode=mybir.MatmulPerfMode.DoubleRow)
                aTs = tpool.tile([P, U], dt.bfloat16, tag="aTs", bufs=2)
                nc.scalar.activation(aTs[:], aT[:], AF.Tanh, bias=b1_sb[:, 0:1],
                                     scale=1.0 / W1S)
                aTs_tiles[t] = aTs

            def lps_tile(t):
                """second attn matmul + exp (one tile behind attn1 so the
                tanh has matmuls to hide behind)."""
                k, r = divmod(t, R)
                lps = psA.tile([P, U], dt.float32, tag="psAt")
                nc.tensor.matmul(lps[:], w2r_sb[:], aTs_tiles.pop(t)[:],
                                 start=True, stop=True)
                nc.scalar.activation(
                    e_sb[:, k, r], lps[:], AF.Exp, bias=b2_sb[:, 0:1],
                    accum_out=esum_sb[:, t:t + 1])

            def embed_tile(t):
                k, r = divmod(t, R)
                xt = xt_tiles.pop(t)
                for kc in range(KC[r]):
                    ko = (OFFC[r] + kc) * P
                    hp = psA.tile([P, U], dt.float32, tag="psAe")
                    for c in range(DC):
                        nc.tensor.matmul(hp[:], we_sb[:, c, ko:ko + P], xt[:, c],
                                         start=(c == 0), stop=(c == DC - 1))
                    nc.vector.tensor_copy(h_sb[:, k, OFFC[r] + kc], hp[:])

            def issue_collective():
                # payload: all 16 raw per-(k,r) sums; the pair partner holds
                # the other half of this batch's tokens.  Reduction to the
                # single denominator happens post-collective.
                nc.gpsimd.dma_start(bounce_in[0:1, 0:NT], esum_sb[0:1, 0:NT])
                nc.gpsimd.collective_compute(
                    "AllReduce", ALU.add,
                    ins=[bounce_in[:]],
                    outs=[bounce_out[:]],
                    replica_groups=[[2 * i, 2 * i + 1] for i in range(N_CORES // 2)],
                )

            gT_tiles = {}

            def den_phase():
                nc.scalar.dma_start(den_sb[0:1, 0:NT], bounce_out[0:1, 0:NT])
                nc.vector.tensor_reduce(
                    den_sb[0:1, NT:NT + 1], den_sb[0:1, 0:NT],
                    axis=mybir.AxisListType.X, op=ALU.add)
                nc.vector.reciprocal(den_sb[0:1, NT + 1:NT + 2],
                                     den_sb[0:1, NT:NT + 1])
                nc.gpsimd.partition_broadcast(
                    inv_bc[:, 0:1], den_sb[0:1, NT + 1:NT + 2])

            def gelu_phase(k):
                gT = gpool.tile([P, SKC, U], dt.bfloat16, tag="gT")
                gT_tiles[k] = gT
                for r in range(R):
                    if KC[r] == 0:
                        continue
                    st = tpool.tile([P, U], dt.float32, tag="st", bufs=2)
                    nc.scalar.activation(st[:], e_sb[:, k, r], AF.Identity,
                                         bias=1.0, scale=inv_bc[:, 0:1])
                    for kc in range(KC[r]):
                        ci = OFFC[r] + kc
                        tmp = tpool.tile([P, U], dt.bfloat16, tag="tmp")
                        nc.vector.tensor_tensor(tmp[:], h_sb[:, k, ci], st[:],
                                                ALU.mult)
                        nc.scalar.activation(
                            gT[:, ci], tmp[:], AF.Gelu,
                            bias=be_sb[:, ci:ci + 1] if USE_BE else 0.0)

            def down_phase(k):
                gT = gT_tiles.pop(k)
                for u in range(U // P):
                    ob = opool.tile([P, D], dt.float32, tag="ob")
                    po = psO.tile([P, 2, D // 2], dt.float32, tag="psO")
                    i = 0
                    for r in range(R):
                        for kc in range(KC[r]):
                            ci = OFFC[r] + kc
                            for dn in range(2):
                                nc.tensor.matmul(
                                    po[:, dn], gT[:, ci, u * P:(u + 1) * P],
                                    wd_sb[:, ci, dn * (D // 2):(dn + 1) * (D // 2)],
                                    start=(i == 0), stop=(i == SKC - 1))
                            i += 1
                    if USE_BD:
                        nc.vector.tensor_tensor(
                            ob[:], po[:].rearrange("p a b -> p (a b)"),
                            bd_sb[:], ALU.add)
                    else:
                        nc.vector.tensor_copy(
                            ob[:], po[:].rearrange("p a b -> p (a b)"))
                    nc.sync.dma_start(outp[k, u * P:(u + 1) * P, :], ob[:])

            # ---- prologue: first R tiles' attention (fp8), clock heater.
            # ALL fp8 tile loads are issued here on the gpsimd queue; the
            # pool's WAR semaphores self-pace them behind the attn stream.
            for t in range(R):
                load_x8(t)
            load_x(0)
            nc.scalar.dma_start(be_sb[:], bep[:, :])
            load_we(0)
            for t in range(R, NT):
                load_x8(t)
            for t in range(R):
                if 1 <= t < R:
                    load_we(t)
                attn1_tile(t)
                heater(16)
                if t:
                    lps_tile(t - 1)
            lps_tile(R - 1)

            # ---- main loop: attention (fp8, 2 tiles/iter) leads; embed
            # streams behind on the bf16 tiles ----
            loaded = 1
            for t in range(NT):
                while loaded < min(NT, t + XPOOL - 1):
                    load_x(loaded)
                    loaded += 1
                embed_tile(t)
                for j in sched[t]:
                    attn1_tile(j)
                    if j - 1 >= R:
                        lps_tile(j - 1)
                if sched[t] and sched[t][-1] == NT - 1:
                    lps_tile(NT - 1)
                    issue_collective()
                if t == 9:
                    # stage-2 weights: off the x-load critical window
                    nc.scalar.dma_start(
                        wd_sb[:], wdp.ap().rearrange("(c p) n -> p c n", p=P))
                    nc.scalar.dma_start(bd_sb[:], bdp[:, :])

            # ---- stage 2, software-pipelined ----
            den_phase()
            gelu_phase(0)
            gelu_phase(1)
            down_phase(0)
            gelu_phase(2)
            down_phase(1)
            gelu_phase(3)
            down_phase(2)
            down_phase(3)

    nc.compile()
    return nc


def _run(inputs, trace=False, trace_cores=None):
    from concourse.bass_utils import run_bass_kernel_spmd

    x_pre, x8_pre, weights, meta = _host_prep(**inputs)
    nc = _build(meta)
    in_maps = [dict(x=np.ascontiguousarray(x_pre[c]),
                    x8=np.ascontiguousarray(x8_pre[c]), **weights)
               for c in range(N_CORES)]
    kw = {}
    if trace_cores is not None:
        kw["trace_cores"] = trace_cores
    res = run_bass_kernel_spmd(nc, in_maps, core_ids=list(range(N_CORES)),
                               trace=trace, **kw)
    out = np.empty((B, T // R, D), dtype=np.float32)
    for c in range(N_CORES):
        b, h = divmod(c, 2)
        out[b, h * K * U:(h + 1) * K * U, :] = (
            res.results[c]["out"].reshape(K * U, D))
    return out, res


def kernel(**inputs):
    out, _ = _run(inputs, trace=False)
    return out


# revision 26
# speedup vs baseline: 1.1497x; 1.1497x over previous
"""Trainium2 Bass kernel for nn_ADS_30313879175331.

Pipeline (reference):
  attn-softmax pooling over T -> x *= (1+aw) -> shuffle tokens by perm
  -> Linear(D,D)+GELU -> rearrange (B,T/4,4,D)->(B,T/4,D*4)
  -> gather keep_idx columns -> Linear(D,D) -> (B, T/4, D)

Device strategy (8 cores, PAIR-sharded):
  * Core c handles batch b=c//2, permuted-token half h=c%2 (8192 tokens),
    i.e. output rows [h*2048, (h+1)*2048) of batch b.  The softmax
    denominator for batch b then only needs a 2-core AllReduce within the
    pair (groups [[0,1],[2,3],[4,5],[6,7]]) instead of an 8-core mesh --
    inter-core arrival skew no longer serializes the kernel.
  * Host folds perm + the (rearrange+keep_idx gather) into pure data layout:
    tokens grouped per (core, u-block k, class r = shuffled_pos % 4); embed
    weight columns {d : 4d+r in keep_idx} and matching w_down rows are
    pre-selected per class, so the device kernel is fully dense.
  * Three phases, kept deliberately decoupled so no in-order queue ever
    blocks across streams:
    - Phase A: attention over all 16 (k,r) tiles from a small fp8 copy of
      x (8.4 MB; fp8 DoubleRow matmuls at 2x PE rate; w1 pre-scaled by 64
      into fp8 range, compensated in the tanh scale).  Heater matmuls pin
      the HAM clock; the single pair AllReduce (16 partial sums) triggers
      at ~30us.  Attention errors only perturb softmax logits (~5e-3 abs
      on |logit|~0.1) and aw itself is O(1e-4) -- harmless.
    - Phase B: embed matmuls h = x @ We (bf16) over all tiles; a few run
      inline at the end of phase A as their bf16 tiles land.
    - Phase C: s = 1 + e/den ; g = gelu(h*s) ; out = sum g @ Wd + b_down,
      software-pipelined (gelu k+1 overlaps down k).  Partial 128-chunks
      of the keep-columns (class tails) are DMA-packed into full chunks
      between gelu and down, so down contracts over ceil(1024/128)=8
      chunks instead of sum(ceil(Kr/128)).
  All heavy matmuls bf16 with f32 PSUM accumulation.
"""

import numpy as np
import ml_dtypes

B, T, D, ATTN, R = 4, 16384, 1024, 128, 4
N_CORES = 8
K = 4                       # u-blocks per core = 4 x 512 rows = 2048 rows
U = 512                     # tokens per (k,r) tile / output rows per block
DC = D // 128               # contraction chunks over D = 8
P = 128
NT = K * R                  # 16 tiles per core
XPOOL = 6                   # bf16 x tile buffers resident in SBUF (embed)
X8POOL = 6                  # fp8 x tile buffers (attention)
W1S = 64.0                  # fp8 range pre-scale for w_attn1

_BF16 = ml_dtypes.bfloat16
_FP8 = ml_dtypes.float8_e4m3fn


def _host_prep(x, w_attn1, b_attn1, w_attn2, b_attn2,
               w_embed, b_embed, w_down, b_down, perm, keep_idx):
    """Pure-layout host work: sharding, permutation gather, weight selection."""
    perm = np.asarray(perm).astype(np.int64)
    keep = np.asarray(keep_idx).astype(np.int64)
    x = np.asarray(x, dtype=np.float32)

    # class split of keep_idx (duplicates preserved, order by j)
    cols, rows = [], []
    for r in range(R):
        sel = np.nonzero((keep % R) == r)[0]
        rows.append(sel)                  # indices j into w_down rows
        cols.append(keep[sel] // R)       # embed output columns d
    Kr = [len(c) for c in cols]
    KC = [(k + P - 1) // P for k in Kr]   # 128-chunks per class (may be 0)
    SKC = sum(KC)
    SKP = SKC * P
    OFFC = np.concatenate([[0], np.cumsum(KC)]).astype(int)  # chunk offsets

    f32 = np.float32
    we = np.zeros((D, SKP), dtype=f32)
    be = np.zeros((SKP,), dtype=f32)
    rows_by_ci = {}
    for r in range(R):
        o = OFFC[r] * P
        if Kr[r]:
            we[:, o:o + Kr[r]] = np.asarray(w_embed, f32)[:, cols[r]]
            be[o:o + Kr[r]] = np.asarray(b_embed, f32)[cols[r]]
        for i in range(KC[r]):
            rows_by_ci[OFFC[r] + i] = rows[r][i * P:(i + 1) * P]
    be_pc = be.reshape(SKC, P).T.copy()                       # (128, SKC)

    # ---- down-side packed contraction: merge partial tail chunks ----
    fulls, tails = [], []
    for r in range(R):
        for i in range(KC[r]):
            ci = OFFC[r] + i
            used = min(P, Kr[r] - i * P)
            if used == P:
                fulls.append(int(ci))
            else:
                tails.append((int(ci), int(used)))
    tails.sort(key=lambda t: -t[1])
    bins = []                             # [ [pieces=(ci,used,dst_lo)], tot ]
    for ci, used in tails:
        for b in bins:
            if b[1] + used <= P:
                b[0].append((ci, used, b[1]))
                b[1] += used
                break
        else:
            bins.append([[(ci, used, 0)], used])
    NDC = len(fulls) + len(bins)

    wd_src = np.asarray(w_down, f32)
    wd_p = np.zeros((NDC * P, D), dtype=f32)
    for dci, ci in enumerate(fulls):
        wd_p[dci * P:(dci + 1) * P] = wd_src[rows_by_ci[ci], :]
    for bi, (pieces, _tot) in enumerate(bins):
        base = (len(fulls) + bi) * P
        for ci, used, dst_lo in pieces:
            wd_p[base + dst_lo:base + dst_lo + used] = \
                wd_src[rows_by_ci[ci][:used], :]

    w1 = (np.asarray(w_attn1, f32) * W1S).astype(_FP8)        # (D, ATTN)
    w2r = np.tile(np.asarray(w_attn2, f32).reshape(ATTN, 1), (1, P)).astype(_BF16)
    b1 = np.asarray(b_attn1, f32).reshape(ATTN, 1)
    b2 = np.full((P, 1), float(np.asarray(b_attn2, f32).reshape(-1)[0]), f32)
    bd = np.broadcast_to(np.asarray(b_down, f32), (P, D)).astype(_BF16)

    # x gather per core: core c = (batch b=c//2, half h=c%2).
    # x_pre[c][k, r, d, u] = x[b, perm[h*8192 + k*2048 + 4u + r], d]
    pidx = perm.reshape(2, K, U, R)                           # [h, k, u, r]
    g = x[:, pidx, :]                                         # (B, 2, K, U, R, D)
    x_pre, x8_pre = [], []
    for c in range(N_CORES):
        arr = np.ascontiguousarray(
            g[c // 2, c % 2].transpose(0, 2, 3, 1))           # (K, R, D, U)
        x_pre.append(arr.astype(_BF16))
        x8_pre.append(arr.astype(_FP8))

    meta = dict(Kr=Kr, KC=KC, SKC=SKC, SKP=SKP, OFFC=OFFC,
                fulls=fulls, bins=bins, NDC=NDC,
                use_bd=bool(np.any(np.asarray(b_down))),
                use_be=bool(np.any(np.asarray(b_embed))))
    weights = dict(
        w1=w1, w2r=w2r, b1=b1, b2=b2, bd=bd,
        we=we.astype(_BF16), wd=wd_p.astype(_BF16), be=be_pc,
    )
    return x_pre, x8_pre, weights, meta


def _build(meta):
    import concourse.bacc as bacc
    import concourse.mybir as mybir
    import concourse.tile as tile

    dt = mybir.dt
    AF = mybir.ActivationFunctionType
    ALU = mybir.AluOpType
    KC, SKC, SKP, OFFC = meta["KC"], meta["SKC"], meta["SKP"], meta["OFFC"]
    fulls, bins, NDC = meta["fulls"], meta["bins"], meta["NDC"]
    NB = len(bins)
    USE_BD = meta["use_bd"]
    USE_BE = meta["use_be"]

    nc = bacc.Bacc(None, target_bir_lowering=False, debug=False,
                   num_devices=N_CORES)

    xp = nc.declare_dram_parameter("x", [K, R, D, U], dt.bfloat16, isOutput=False)
    x8p = nc.declare_dram_parameter("x8", [K, R, D, U], dt.float8e4, isOutput=False)
    w1p = nc.declare_dram_parameter("w1", [D, ATTN], dt.float8e4, isOutput=False)
    w2p = nc.declare_dram_parameter("w2r", [ATTN, P], dt.bfloat16, isOutput=False)
    wep = nc.declare_dram_parameter("we", [D, SKP], dt.bfloat16, isOutput=False)
    wdp = nc.declare_dram_parameter("wd", [NDC * P, D], dt.bfloat16, isOutput=False)
    bep = nc.declare_dram_parameter("be", [P, SKC], dt.float32, isOutput=False)
    b1p = nc.declare_dram_parameter("b1", [ATTN, 1], dt.float32, isOutput=False)
    b2p = nc.declare_dram_parameter("b2", [P, 1], dt.float32, isOutput=False)
    bdp = nc.declare_dram_parameter("bd", [P, D], dt.bfloat16, isOutput=False)
    outp = nc.declare_dram_parameter("out", [K, U, D], dt.float32, isOutput=True)

    with tile.TileContext(nc) as tc:
        with (
            tc.tile_pool(name="const", bufs=1) as cpool,
            tc.tile_pool(name="xin", bufs=XPOOL) as xpool,
            tc.tile_pool(name="x8in", bufs=X8POOL) as x8pool,
            tc.tile_pool(name="gts", bufs=2) as gpool,
            tc.tile_pool(name="outs", bufs=2) as opool,
            tc.tile_pool(name="tmps", bufs=3) as tpool,
            tc.tile_pool(name="psA", bufs=2, space="PSUM") as psA,
            tc.tile_pool(name="psO", bufs=2, space="PSUM") as psO,  # 2-bank tiles
            tc.tile_pool(name="dram", bufs=1, space="DRAM") as dram,
        ):
            w1_sb = cpool.tile([P, DC, ATTN], dt.float8e4)
            w2r_sb = cpool.tile([P, P], dt.bfloat16)
            b1_sb = cpool.tile([ATTN, 1], dt.float32)
            b2_sb = cpool.tile([P, 1], dt.float32)
            be_sb = cpool.tile([P, SKC], dt.float32)
            we_sb = cpool.tile([P, DC, SKP], dt.bfloat16)
            bd_sb = cpool.tile([P, D], dt.bfloat16)
            wd_sb = cpool.tile([P, NDC, D], dt.bfloat16)
            _we_ap = wep.ap().rearrange("(c p) k -> p c k", p=P)

            def load_we(_r):
                _a, _b = OFFC[_r] * P, OFFC[_r + 1] * P
                if _a < _b:
                    nc.scalar.dma_start(we_sb[:, :, _a:_b], _we_ap[:, :, _a:_b])

            e_sb = cpool.tile([P, K, R, U], dt.float8e4)    # exp(logits), bcast rows
            esum_sb = cpool.tile([P, NT], dt.float32)       # per-(k,r) local sums
            h_sb = cpool.tile([P, K, SKC, U], dt.bfloat16)  # x @ We (transposed)
            inv_bc = cpool.tile([P, 1], dt.float32)         # 1/denominator bcast
            den_sb = cpool.tile([1, NT + 8], dt.float32)    # raw sums + reduced

            bounce_in = dram.tile([1, NT], dt.float32, name="cc_in")
            # NOTE: Shared outputs need >4-core groups; pairs use Local.
            bounce_out = dram.tile([1, NT], dt.float32, name="cc_out")

            xt_tiles, x8_tiles, aTs_tiles = {}, {}, {}

            def heater(n):
                # dummy matmuls on resident weights: keeps the PE HAM window
                # busy across startup DMA gaps so the clock stays at 2.4GHz
                hps = psA.tile([P, 64], dt.float32, tag="psAt")
                for i in range(n):
                    nc.tensor.matmul(hps[:], w2r_sb[:], w2r_sb[:, :64],
                                     start=(i == 0), stop=(i == n - 1))
                nc.vector.tensor_copy(den_sb[0:1, NT + 6:NT + 7], hps[0:1, 0:1])

            def load_x(t, eng):
                k, r = divmod(t, R)
                xt = xpool.tile([P, DC, U], dt.bfloat16, tag="xt", name=f"xt{t}")
                src = xp[k, r].rearrange("(c p) u -> p c u", p=P)
                eng.dma_start(xt[:], src)
                xt_tiles[t] = xt

            def load_x8(t):
                # attention stream: sync queue, issued upfront (phase A's
                # only DMA dependency)
                k, r = divmod(t, R)
                xt = x8pool.tile([P, DC, U], dt.float8e4, tag="x8", name=f"x8t{t}")
                src = x8p[k, r].rearrange("(c p) u -> p c u", p=P)
                if t == 0:
                    for c in range(DC):
                        nc.sync.dma_start(xt[:, c], src[:, c])
                else:
                    nc.sync.dma_start(xt[:], src)
                x8_tiles[t] = xt

            def attn1_tile(t):
                # fp8 DoubleRow: contraction 256 per instruction, 2x PE rate
                xt = x8_tiles.pop(t)
                aT = psA.tile([P, U], dt.float32, tag="psAt")
                for c in range(DC // 2):
                    nc.tensor.matmul(aT[:], w1_sb[:, 2 * c:2 * c + 2, :],
                                     xt[:, 2 * c:2 * c + 2, :],
                                     start=(c == 0), stop=(c == DC // 2 - 1),
                                     perf_mode=mybir.MatmulPerfMode.DoubleRow)
                aTs = tpool.tile([P, U], dt.bfloat16, tag="aTs", bufs=2)
                nc.scalar.activation(aTs[:], aT[:], AF.Tanh, bias=b1_sb[:, 0:1],
                                     scale=1.0 / W1S)
                aTs_tiles[t] = aTs

            def lps_tile(t):
                """second attn matmul + exp (one tile behind attn1 so the
                tanh has matmuls to hide behind)."""
                k, r = divmod(t, R)
                lps = psA.tile([P, U], dt.float32, tag="psAt")
                nc.tensor.matmul(lps[:], w2r_sb[:], aTs_tiles.pop(t)[:],
                                 start=True, stop=True)
                nc.scalar.activation(
                    e_sb[:, k, r], lps[:], AF.Exp, bias=b2_sb[:, 0:1],
                    accum_out=esum_sb[:, t:t + 1])

            def embed_tile(t):
                k, r = divmod(t, R)
                xt = xt_tiles.pop(t)
                for kc in range(KC[r]):
                    ko = (OFFC[r] + kc) * P
                    hp = psA.tile([P, U], dt.float32, tag="psAe")
                    for c in range(DC):
                        nc.tensor.matmul(hp[:], we_sb[:, c, ko:ko + P], xt[:, c],
                                         start=(c == 0), stop=(c == DC - 1))
                    nc.vector.tensor_copy(h_sb[:, k, OFFC[r] + kc], hp[:])

            def issue_collective():
                # payload: all 16 raw per-(k,r) sums; the pair partner holds
                # the other half of this batch's tokens.
                nc.gpsimd.dma_start(bounce_in[0:1, 0:NT], esum_sb[0:1, 0:NT])
                nc.gpsimd.collective_compute(
                    "AllReduce", ALU.add,
                    ins=[bounce_in[:]],
                    outs=[bounce_out[:]],
                    replica_groups=[[2 * i, 2 * i + 1] for i in range(N_CORES // 2)],
                )

            gT_tiles, gx_tiles = {}, {}

            def den_phase():
                nc.scalar.dma_start(den_sb[0:1, 0:NT], bounce_out[0:1, 0:NT])
                nc.vector.tensor_reduce(
                    den_sb[0:1, NT:NT + 1], den_sb[0:1, 0:NT],
                    axis=mybir.AxisListType.X, op=ALU.add)
                nc.vector.reciprocal(den_sb[0:1, NT + 1:NT + 2],
                                     den_sb[0:1, NT:NT + 1])
                nc.gpsimd.partition_broadcast(
                    inv_bc[:, 0:1], den_sb[0:1, NT + 1:NT + 2])

            def gelu_phase(k):
                gT = gpool.tile([P, SKC, U], dt.bfloat16, tag="gT")
                gT_tiles[k] = gT
                for r in range(R):
                    if KC[r] == 0:
                        continue
                    st = tpool.tile([P, U], dt.float32, tag="st", bufs=2)
                    nc.scalar.activation(st[:], e_sb[:, k, r], AF.Identity,
                                         bias=1.0, scale=inv_bc[:, 0:1])
                    for kc in range(KC[r]):
                        ci = OFFC[r] + kc
                        tmp = tpool.tile([P, U], dt.bfloat16, tag="tmp")
                        nc.vector.tensor_tensor(tmp[:], h_sb[:, k, ci], st[:],
                                                ALU.mult)
                        nc.scalar.activation(
                            gT[:, ci], tmp[:], AF.Gelu,
                            bias=be_sb[:, ci:ci + 1] if USE_BE else 0.0)
                # pack class-tail chunks into full contraction chunks for
                # the down matmul (partition-moving SBUF->SBUF DMAs)
                if NB:
                    gx = gpool.tile([P, NB, U], dt.bfloat16, tag="gTx", bufs=2)
                    gx_tiles[k] = gx
                    for bi, (pieces, tot) in enumerate(bins):
                        if tot < P:
                            nc.gpsimd.memset(gx[tot:P, bi], 0.0)
                        for ci, used, dst_lo in pieces:
                            nc.sync.dma_start(gx[dst_lo:dst_lo + used, bi],
                                              gT[0:used, ci])

            def down_phase(k):
                gT = gT_tiles.pop(k)
                gx = gx_tiles.pop(k) if NB else None
                for u in range(U // P):
                    ob = opool.tile([P, D], dt.float32, tag="ob")
                    po = psO.tile([P, 2, D // 2], dt.float32, tag="psO")
                    for i in range(NDC):
                        stat = (gT[:, fulls[i], u * P:(u + 1) * P] if i < len(fulls)
                                else gx[:, i - len(fulls), u * P:(u + 1) * P])
                        for dn in range(2):
                            nc.tensor.matmul(
                                po[:, dn], stat,
                                wd_sb[:, i, dn * (D // 2):(dn + 1) * (D // 2)],
                                start=(i == 0), stop=(i == NDC - 1))
                    if USE_BD:
                        nc.vector.tensor_tensor(
                            ob[:], po[:].rearrange("p a b -> p (a b)"),
                            bd_sb[:], ALU.add)
                    else:
                        nc.vector.tensor_copy(
                            ob[:], po[:].rearrange("p a b -> p (a b)"))
                    nc.sync.dma_start(outp[k, u * P:(u + 1) * P, :], ob[:])

            # ================= schedule =================
            # consts: w1 first (attention-critical), rest behind
            nc.scalar.dma_start(w1_sb[:], w1p.ap().rearrange("(c p) a -> p c a", p=P))
            nc.scalar.dma_start(w2r_sb[:], w2p[:, :])
            nc.scalar.dma_start(b1_sb[:], b1p[:, :])
            nc.scalar.dma_start(b2_sb[:], b2p[:, :])
            # phase A's x8 stream: all issued upfront on sync (pool-paced).
            # Embed-side prefetch is staggered through phase A on the gpsimd/
            # scalar queues so it doesn't dilute x8's bandwidth share.
            for t in range(NT):
                load_x8(t)
            nc.scalar.dma_start(be_sb[:], bep[:, :])
            load_we(0)
            load_x(0, nc.gpsimd)
            load_x(1, nc.gpsimd)

            # ---- phase A: attention over all tiles; a few embeds inline ----
            EMB_AT = {6: 0, 10: 1, 14: 2}   # slot -> embed tile
            XB_AT = {8: [2], 12: [3, 4], 14: [5]}
            WE_AT = {4: 1, 8: 2, 10: 3}
            for j in range(NT):
                attn1_tile(j)
                if j:
                    lps_tile(j - 1)
                if j < 12:
                    heater(6)
                if j in WE_AT:
                    load_we(WE_AT[j])
                for t in XB_AT.get(j, ()):
                    load_x(t, nc.gpsimd)
                if j in EMB_AT:
                    embed_tile(EMB_AT[j])
            lps_tile(NT - 1)
            issue_collective()

            # ---- phase B: remaining embeds (PE-bound) ----
            nc.scalar.dma_start(
                wd_sb[:], wdp.ap().rearrange("(c p) n -> p c n", p=P))
            nc.scalar.dma_start(bd_sb[:], bdp[:, :])
            loaded = 6
            for t in range(3, NT):
                while loaded < min(NT, t + XPOOL - 1):
                    load_x(loaded, nc.gpsimd)
                    loaded += 1
                embed_tile(t)

            # ---- phase C: stage 2, software-pipelined ----
            den_phase()
            gelu_phase(0)
            gelu_phase(1)
            down_phase(0)
            gelu_phase(2)
            down_phase(1)
            gelu_phase(3)
            down_phase(2)
            down_phase(3)

    nc.compile()
    return nc


def _run(inputs, trace=False, trace_cores=None):
    from concourse.bass_utils import run_bass_kernel_spmd

    x_pre, x8_pre, weights, meta = _host_prep(**inputs)
    nc = _build(meta)
    in_maps = [dict(x=np.ascontiguousarray(x_pre[c]),
                    x8=np.ascontiguousarray(x8_pre[c]), **weights)
               for c in range(N_CORES)]
    kw = {}
    if trace_cores is not None:
        kw["trace_cores"] = trace_cores
    res = run_bass_kernel_spmd(nc, in_maps, core_ids=list(range(N_CORES)),
                               trace=trace, **kw)
    out = np.empty((B, T // R, D), dtype=np.float32)
    for c in range(N_CORES):
        b, h = divmod(c, 2)
        out[b, h * K * U:(h + 1) * K * U, :] = (
            res.results[c]["out"].reshape(K * U, D))
    return out, res


def kernel(**inputs):
    out, _ = _run(inputs, trace=False)
    return out
